# revision 2
# baseline (speedup 1.0000x reference)
"""GAE (generalized advantage estimation) Trainium2 kernel — bf16 streaming.

Problem: nn_CustomGAE — B=512, T=2048, D=64.
  value = obs @ W + b ; next_value = next_obs @ W + b
  td0 = reward + gamma*nd*next_value - value ; coef = gamma*lambda*nd
  A_t = td0_t + coef_t * A_{t+1}  (reverse scan over T, independent per traj)
  returns (advantage, value_target = advantage + value)

Sharding: pure data parallel over B across 8 cores (64 trajectories/core).
Host pre-swizzles each shard (half, batch)-major so SBUF partition
p = h*64 + b holds timesteps t in [h*1024, (h+1)*1024).

v2 design (HBM-roofline targeted):
  * obs/next_obs are cast to bf16 on the host -> per-core HBM traffic drops
    from ~68.8 MB (f32) to ~35 MB, moving the roofline from ~204us to ~100us
    (HBM-per-NC limit is ~358 GB/s: 716 GB/s/stack shared by 2 cores).
  * The value-head matvec's two free-axis reduces are DVE-only ops; in bf16
    they run in 2x_1P mode (2 elem/cyc) = ~68us total, leaving ~25us DVE
    slack under the DMA floor.
  * The two elementwise multiplies (obs*W, nobs*W) are split BY d-PLANE
    across three engines per chunk, all in-place on the streamed tile:
      - ACT:  per-d activation(Copy, scale=W[d]) on strided columns
      - POOL: one tensor_tensor over its d-range
      - DVE:  one tensor_tensor (2x bf16) over its d-range (the slack)
    The region-based tile tracker sees the writes as disjoint, so the three
    engines run concurrently; the chunk reduce then waits on all three.
  * td0/coef/scan epilogue stays f32 (bf16 coef would bias the geometric
    accumulation); per-chunk epilogue keeps the tail short.
"""

import sys

sys.path.insert(0, "/opt/trn_rl_repo")

from contextlib import ExitStack

import ml_dtypes
import numpy as np

import concourse.bacc as bacc
import concourse.mybir as mybir
import concourse.tile as tile
from concourse.bass_utils import run_bass_kernel_spmd

GAMMA = 0.99
LMBDA = 0.95

B, T, D = 512, 2048, 64
NCORES = 8
BL = B // NCORES  # 64 trajectories per core
H = 2  # trajectory halves stacked on partitions -> 128 partitions
P = H * BL  # 128
F32 = mybir.dt.float32
BF16 = mybir.dt.bfloat16
U8 = mybir.dt.uint8
BF_NP = ml_dtypes.bfloat16

# Results of the last hardware run, for test harnesses.
LAST_RESULTS = None


def _build_iter(
    nc, opool, npool, ppool, dpool, w_bf, w_f, b_t, bnd,
    obs_d, nobs_d, rw_d, dn_d, adv_d, tgt_d, tp, tcs,
    a_act, p_pool, nocompute=False, dual_dma=False, out_scalar=False,
):
    """One full pass: load inputs, matvec, per-chunk epilogue, scan, store.

    tcs: list of chunk sizes (timesteps per partition), summing to tp.
    a_act/p_pool: # of d-planes multiplied by ACT / POOL per tensor per
    chunk; DVE takes the remaining [a_act+p_pool, 64) planes.
    """
    mult = mybir.AluOpType.mult
    add = mybir.AluOpType.add
    sub = mybir.AluOpType.subtract
    eng2 = nc.scalar if dual_dma else nc.sync
    oeng = nc.scalar if out_scalar else nc.sync

    s_dve = a_act + p_pool  # first DVE-owned d-plane
    assert s_dve <= D and a_act % 2 == 0 and p_pool % 2 == 0

    # persistent [P, tp] state
    vb = ppool.tile([P, tp], F32)    # value = obs@W + b
    coef = ppool.tile([P, tp], F32)  # gamma*lambda*nd
    g = ppool.tile([P, tp], F32)     # gamma*nd
    td0 = ppool.tile([P, tp], F32)
    adv = ppool.tile([P, tp], F32)
    tgt = ppool.tile([P, tp], F32)
    rw_t = dpool.tile([P, tp], F32)
    dn_t = dpool.tile([P, tp], U8)

    if nocompute:
        # IO-only build to measure the DMA floor: stream everything, write
        # garbage outputs of the right size.
        off = 0
        for tc in tcs:
            ot = opool.tile([P, tc * D], BF16)
            nt = npool.tile([P, tc * D], BF16)
            fs = slice(off * D, (off + tc) * D)
            nc.sync.dma_start(ot[:], obs_d.ap()[:, fs])
            eng2.dma_start(nt[:], nobs_d.ap()[:, fs])
            off += tc
        nc.sync.dma_start(rw_t[:], rw_d.ap())
        nc.sync.dma_start(dn_t[:], dn_d.ap())
        oeng.dma_start(adv_d.ap(), rw_t[:])
        oeng.dma_start(tgt_d.ap(), rw_t[:])
        return

    first = True
    off = 0
    for j, tc in enumerate(tcs):
        ot = opool.tile([P, tc * D], BF16)
        nt = npool.tile([P, tc * D], BF16)
        fs = slice(off * D, (off + tc) * D)
        cs = slice(off, off + tc)
        nc.sync.dma_start(ot[:], obs_d.ap()[:, fs])
        eng2.dma_start(nt[:], nobs_d.ap()[:, fs])
        if first:
            # queued behind chunk 0 so the stream starts immediately; needed
            # only when chunk-0's epilogue runs, ~one chunk-DMA later.
            nc.sync.dma_start(rw_t[:], rw_d.ap())
            nc.sync.dma_start(dn_t[:], dn_d.ap())
            # nd-derived factors, overlapping the chunk-0 DMA.
            ndf = dpool.tile([P, tp], F32)
            nc.vector.tensor_copy(ndf[:], dn_t[:])  # u8 -> f32
            nc.scalar.activation(
                g[:], ndf[:], mybir.ActivationFunctionType.Copy,
                bias=GAMMA, scale=-GAMMA,
            )
            nc.scalar.activation(
                coef[:], ndf[:], mybir.ActivationFunctionType.Copy,
                bias=GAMMA * LMBDA, scale=-GAMMA * LMBDA,
            )
            first = False

        # ---- matvec: in-place multiply by W, split by d-plane ----
        v_s = dpool.tile([P, tc], BF16)   # obs@W (no bias)
        nv_s = dpool.tile([P, tc], BF16)  # nobs@W (no bias)
        for x3 in (
            ot[:].rearrange("p (t d) -> p t d", d=D),
            nt[:].rearrange("p (t d) -> p t d", d=D),
        ):
            for d in range(a_act):
                nc.scalar.activation(
                    x3[:, :, d : d + 1], x3[:, :, d : d + 1],
                    mybir.ActivationFunctionType.Copy,
                    bias=0.0, scale=w_f[:, d : d + 1],
                )
            if p_pool:
                wbp = (
                    w_bf[:, a_act:s_dve].unsqueeze(1)
                    .broadcast_to([P, tc, p_pool])
                )
                nc.gpsimd.tensor_tensor(
                    out=x3[:, :, a_act:s_dve], in0=x3[:, :, a_act:s_dve],
                    in1=wbp, op=mult,
                )
            if s_dve < D:
                wbv = (
                    w_bf[:, s_dve:D].unsqueeze(1)
                    .broadcast_to([P, tc, D - s_dve])
                )
                nc.vector.tensor_tensor(
                    out=x3[:, :, s_dve:D], in0=x3[:, :, s_dve:D],
                    in1=wbv, op=mult,
                )
        with nc.allow_low_precision("bf16 matvec; tolerance 2e-2"):
            nc.vector.tensor_reduce(
                out=v_s[:], in_=ot[:].rearrange("p (t d) -> p t d", d=D),
                axis=mybir.AxisListType.X, op=add,
            )
            nc.vector.tensor_reduce(
                out=nv_s[:], in_=nt[:].rearrange("p (t d) -> p t d", d=D),
                axis=mybir.AxisListType.X, op=add,
            )

        # ---- per-chunk epilogue: td0 = rw + g*(nv+b) - (v+b) ----
        nc.vector.tensor_scalar_add(vb[:, cs], v_s[:], b_t[:, 0:1])
        nvb = dpool.tile([P, tc], F32)
        nc.vector.tensor_scalar_add(nvb[:], nv_s[:], b_t[:, 0:1])
        q = dpool.tile([P, tc], F32)
        nc.vector.tensor_tensor(out=q[:], in0=g[:, cs], in1=nvb[:], op=mult)
        s_t = dpool.tile([P, tc], F32)
        nc.vector.tensor_tensor(out=s_t[:], in0=rw_t[:, cs], in1=vb[:, cs], op=sub)
        nc.vector.tensor_tensor(out=td0[:, cs], in0=q[:], in1=s_t[:], op=add)
        off += tc

    # ---- backward scan: second half (later timesteps) first ----
    hi = slice(BL, 2 * BL)
    lo = slice(0, BL)
    nc.vector.tensor_tensor_scan(
        out=adv[hi, ::-1], data0=coef[hi, ::-1], data1=td0[hi, ::-1],
        initial=0.0, op0=mult, op1=add,
    )
    nc.vector.tensor_tensor(out=tgt[hi, :], in0=adv[hi, :], in1=vb[hi, :], op=add)
    oeng.dma_start(bnd[:], adv[hi, 0:1])
    oeng.dma_start(adv_d.ap()[hi, :], adv[hi, :])
    oeng.dma_start(tgt_d.ap()[hi, :], tgt[hi, :])
    nc.vector.tensor_tensor_scan(
        out=adv[lo, ::-1], data0=coef[lo, ::-1], data1=td0[lo, ::-1],
        initial=bnd[:, 0:1], op0=mult, op1=add,
    )
    nc.vector.tensor_tensor(out=tgt[lo, :], in0=adv[lo, :], in1=vb[lo, :], op=add)
    oeng.dma_start(adv_d.ap()[lo, :], adv[lo, :])
    oeng.dma_start(tgt_d.ap()[lo, :], tgt[lo, :])


def build_program(
    t_total=T, tcs=None, repeat=1, nocompute=False, bench_internal=False,
    a_act=28, p_pool=20, obufs=2, nbufs=2, dbl=2, dual_dma=False,
    out_scalar=False,
):
    """Build the per-core Bass program (all 8 cores run it SPMD on their own
    shard). DRAM layouts are (half, batch)-major as produced by shard_inputs.
    repeat>1 re-runs the pipeline inside one NEFF for delta-timing;
    bench_internal makes obs/nobs Internal DRAM (garbage values, not shipped
    per call) so benchmark invocations are cheap."""
    tp = t_total // H  # timesteps per partition
    if tcs is None:
        ntc = max(1, tp // 256)
        tcs = [tp // ntc] * ntc
    assert sum(tcs) == tp

    nc = bacc.Bacc(
        "TRN2", target_bir_lowering=False, debug=False, enable_asserts=False
    )

    big_kind = "Internal" if bench_internal else "ExternalInput"
    obs_d = nc.dram_tensor("obs", [P, tp * D], BF16, kind=big_kind)
    nobs_d = nc.dram_tensor("nobs", [P, tp * D], BF16, kind=big_kind)
    rw_d = nc.dram_tensor("rw", [P, tp], F32, kind="ExternalInput")
    dn_d = nc.dram_tensor("dn", [P, tp], U8, kind="ExternalInput")
    wbf_d = nc.dram_tensor("wbf", [D], BF16, kind="ExternalInput")
    wf_d = nc.dram_tensor("wf", [D], F32, kind="ExternalInput")
    b_d = nc.dram_tensor("b", [1], F32, kind="ExternalInput")
    adv_d = nc.dram_tensor("adv", [P, tp], F32, kind="ExternalOutput")
    tgt_d = nc.dram_tensor("tgt", [P, tp], F32, kind="ExternalOutput")

    with tile.TileContext(nc) as tc_ctx, ExitStack() as ctx:
        cpool = ctx.enter_context(tc_ctx.tile_pool(name="const", bufs=1))
        opool = ctx.enter_context(tc_ctx.tile_pool(name="obs", bufs=obufs))
        npool = ctx.enter_context(tc_ctx.tile_pool(name="nobs", bufs=nbufs))
        ppool = ctx.enter_context(tc_ctx.tile_pool(name="pers", bufs=1))
        dpool = ctx.enter_context(tc_ctx.tile_pool(name="dbl", bufs=dbl))

        # Value-head weights replicated to every partition.
        w_bf = cpool.tile([P, D], BF16)
        nc.sync.dma_start(w_bf[:], wbf_d.ap().unsqueeze(0).broadcast_to([P, D]))
        w_f = cpool.tile([P, D], F32)
        nc.sync.dma_start(w_f[:], wf_d.ap().unsqueeze(0).broadcast_to([P, D]))
        b_t = cpool.tile([P, 1], F32)
        nc.sync.dma_start(b_t[:], b_d.ap().unsqueeze(0).broadcast_to([P, 1]))

        bnd = cpool.tile([BL, 1], F32)

        for _rep in range(repeat):
            _build_iter(
                nc, opool, npool, ppool, dpool, w_bf, w_f, b_t, bnd,
                obs_d, nobs_d, rw_d, dn_d, adv_d, tgt_d, tp, tcs,
                a_act, p_pool, nocompute=nocompute, dual_dma=dual_dma,
                out_scalar=out_scalar,
            )

    nc.finalize()
    return nc


_NC_CACHE = None


def _get_nc():
    global _NC_CACHE
    if _NC_CACHE is None:
        _NC_CACHE = build_program()
    return _NC_CACHE


def _hmajor(x, tp_cols):
    """[BL, H*tp_cols] row-major -> [H*BL, tp_cols] with row p = h*BL + b."""
    return np.ascontiguousarray(
        x.reshape(BL, H, tp_cols).transpose(1, 0, 2).reshape(H * BL, tp_cols)
    )


def _unhmajor(y):
    """Inverse of _hmajor for outputs: [H*BL, tp] -> [BL, H*tp]."""
    tp = y.shape[1]
    return y.reshape(H, BL, tp).transpose(1, 0, 2).reshape(BL, H * tp)


def shard_inputs(obs, next_obs, reward, done, W, b):
    """Split full inputs into the 8 per-core input maps ((h,b)-major).

    obs/next_obs ship as bf16 (host-side round-to-nearest cast)."""
    obs = np.asarray(obs, dtype=np.float32).reshape(B, T * D).astype(BF_NP)
    nobs = np.asarray(next_obs, dtype=np.float32).reshape(B, T * D).astype(BF_NP)
    rw = np.asarray(reward, dtype=np.float32).reshape(B, T)
    dn = np.asarray(done).astype(np.uint8, copy=False).reshape(B, T)
    w_np = np.ascontiguousarray(np.asarray(W, dtype=np.float32)).reshape(D)
    b_np = np.ascontiguousarray(np.asarray(b, dtype=np.float32)).reshape(1)

    tpd = (T // H) * D
    tp = T // H
    in_maps = []
    for i in range(NCORES):
        sl = slice(i * BL, (i + 1) * BL)
        in_maps.append(
            {
                "obs": _hmajor(obs[sl], tpd),
                "nobs": _hmajor(nobs[sl], tpd),
                "rw": _hmajor(rw[sl], tp),
                "dn": _hmajor(dn[sl], tp),
                "wbf": w_np.astype(BF_NP),
                "wf": w_np,
                "b": b_np,
            }
        )
    return in_maps


def gather_outputs(results):
    advantage = np.concatenate(
        [_unhmajor(r["adv"]) for r in results], axis=0
    ).reshape(B, T, 1)
    value_target = np.concatenate(
        [_unhmajor(r["tgt"]) for r in results], axis=0
    ).reshape(B, T, 1)
    return advantage, value_target


def kernel(obs, next_obs, reward, done, W, b):
    global LAST_RESULTS
    nc = _get_nc()
    in_maps = shard_inputs(obs, next_obs, reward, done, W, b)
    res = run_bass_kernel_spmd(nc, in_maps, core_ids=list(range(NCORES)))
    LAST_RESULTS = res
    return gather_outputs(res.results)
